# revision 1
# baseline (speedup 1.0000x reference)
"""Trainium2 Bass kernel for nn_Density_loss (weighted-kNN hinge loss).

Math: wd[i,j] = sqrt(d2[i,j]) * swn[i] * twn[j], loss = mean(relu(top5min(wd) - 0.01)).
With a_i = swn_i^2, b_j = twn_j^2 (both >= 0): selection of the 5 smallest wd within
a row is invariant to the per-row factor a_i, so we select on tw_j^2 * d2 and apply
a_i and the global normalization factors afterwards.  One augmented bf16 matmul
puts the (negated, tw^2-weighted) squared distances directly in PSUM:
    Saug[i] = [2*s_i, -|s_i|^2, -1]
    Taug[j] = [q_j*t_j, q_j, q_j*|t_j|^2]          (q_j = tw_j^2)
    Saug[i] . Taug[j] = -q_j (|s|^2 + |t|^2 - 2 s.t) = -q_j d2[i,j]
Top-5 smallest per source row == top-5 largest of PSUM -> DVE max8 per PSUM pair,
merged with a second max8.  Then vals = sqrt(scale_i * (-x)) via the Sqrt
activation's per-partition scale (scale_i = (swn_i * rt)^2 restores the exact
weighting), hinge-relu with accumulate, row sums.

Operand layout (d-major for the PE) is produced by casting/scaling tiles to bf16
on the scalar engine, bouncing them through DRAM, and DMA-transposing large
[rows,128] chunks (amortizes per-transpose overhead).  Augmentation rows are
batched into one PE transpose per row group.

Engine assignment (chosen to avoid head-of-line blocking on any sequencer):
  gpsimd: f32 tile loads + bf16 DRAM writes (writes for group g are issued after
          the loads of group g+1, so their waits are already satisfied) + tiny
          tensor ops for the augmentation rows
  scalar: squares (norm accumulation), scale-casts, aug-row copies, finalize
  sync:   DMA transposes, vector loads, output stores
  vector: max8 top-k only
  tensor: matmuls

Sharding: source rows split across 8 cores (1024 rows each); target replicated.
Each core returns per-row hinge sums; host sums and divides by N*k.
"""

import os
import sys

for _p in ("/root/.axon_site/_ro/trn_rl_repo", "/opt/trn_rl_repo"):
    if os.path.isdir(_p):
        if _p not in sys.path:
            sys.path.insert(0, _p)
        break

import numpy as np

N, M, D = 8192, 8192, 512
NCORES = 8
NSH = N // NCORES            # 1024 source rows per core
ITILES = NSH // 128          # 8
JTILES = M // 128            # 64
NJB = 16                     # j-blocks of 512 (matmul moving free dim)
KT = D // 128                # 4 data k-tiles
GROUP_JBS = [2, 2, 2, 2, 2, 2, 2, 2]  # j-blocks per transpose group (sums to NJB)
PSUM_PAIRS = True            # pair two 512-col j-blocks per PSUM max8
PE_T_GROUPS = 2              # leading T groups transposed on the PE (short prologue)
REPEAT = 1                   # repeat the heavy body (timing experiments only)
TOPK = 5
HINGE = 0.01
EPS = 1e-8

_CACHE = {}


def _build():
    from concourse import bacc
    import concourse.mybir as mybir
    from concourse.tile import TileContext
    from concourse.masks import make_identity

    F32 = mybir.dt.float32
    BF16 = mybir.dt.bfloat16
    AF = mybir.ActivationFunctionType
    AX = mybir.AxisListType

    nc = bacc.Bacc("TRN2", target_bir_lowering=False, debug=False,
                   num_devices=NCORES)

    src = nc.dram_tensor("src", [NSH, D], F32, kind="ExternalInput").ap()
    sw = nc.dram_tensor("sw", [NSH], F32, kind="ExternalInput").ap()
    swf = nc.dram_tensor("swf", [N], F32, kind="ExternalInput").ap()
    tgt = nc.dram_tensor("tgt", [M, D], F32, kind="ExternalInput").ap()
    tw = nc.dram_tensor("tw", [M], F32, kind="ExternalInput").ap()
    out = nc.dram_tensor("partial", [ITILES, 128], F32,
                         kind="ExternalOutput").ap()

    with TileContext(nc) as tc:
        with (
            tc.tile_pool(name="const", bufs=1) as const,
            tc.tile_pool(name="smalls", bufs=8) as smalls,
            tc.tile_pool(name="stage", bufs=10) as stage,
            tc.tile_pool(name="sqsc", bufs=4) as sqsc,
            tc.tile_pool(name="pre", bufs=6) as pre,
            tc.tile_pool(name="sTp", bufs=1) as sTp,
            tc.tile_pool(name="tTp", bufs=2) as tTp,
            tc.tile_pool(name="mbp", bufs=1) as mbp,
            tc.tile_pool(name="fin", bufs=4) as fin,
            tc.tile_pool(name="psum", bufs=3 if PSUM_PAIRS else 6,
                         space="PSUM") as psum,
            tc.tile_pool(name="pstr", bufs=2, space="PSUM") as pstr,
            tc.tile_pool(name="dram", bufs=1, space="DRAM") as dram,
        ):
            # ---------- constants ----------
            ones_col = const.tile([128, 1], F32, tag="ones_col")
            nc.vector.memset(ones_col, 1.0)
            ones_row = const.tile([1, 128], F32, tag="ones_row")
            nc.vector.memset(ones_row, 1.0)
            hbias = const.tile([128, 1], F32, tag="hbias")
            nc.vector.memset(hbias, -HINGE)
            ident = const.tile([128, 128], BF16, tag="ident")
            make_identity(nc, ident)
            identf = const.tile([128, 128], F32, tag="identf")
            make_identity(nc, identf)

            def load_colmajor(vec, cols, tag):
                """Load vec[cols*128] as [128, cols] (partition-major) via a
                contiguous load + PE transpose (avoids a strided DMA)."""
                raw = smalls.tile([cols, 128], F32, tag=f"{tag}_raw")
                nc.sync.dma_start(out=raw,
                                  in_=vec.rearrange("(c p) -> c p", p=128))
                pst = pstr.tile([128, cols], F32, tag="pst", name=f"{tag}_pst")
                nc.tensor.transpose(pst, raw, identf[0:cols, 0:cols])
                sb = const.tile([128, cols], F32, tag=tag)
                nc.scalar.copy(out=sb, in_=pst)
                return sb

            # q_j = tw_j^2; the global (rs*rt)^2 factor is applied in finalize
            twsb = load_colmajor(tw, JTILES, "twsb")
            bb_all = const.tile([128, JTILES], F32, tag="bb_all")
            nc.vector.tensor_mul(bb_all, twsb, twsb)

            for _rep in range(REPEAT):
              # ---------- S side: data cast (x2) + aug rows ----------
              saugT = sTp.tile([2, ITILES * 128], BF16, tag="saugT", name="saugT")
              # (b, bn) pairs live at 32-aligned columns so the post-transpose
              # PSUM reads start at partitions 0/32/64/96 (hw alignment rule)
              scmb = const.tile([128, 256], BF16, tag="scmb")
              nc.vector.memset(scmb, 0.0)
              for it in range(ITILES):
                  q = it % 4
                  nc.vector.memset(scmb[:, (it // 4) * 128 + 32 * q + 1:
                                        (it // 4) * 128 + 32 * q + 2], -1.0)

              sT = [sTp.tile([128, ITILES * 128], BF16, tag=f"sT{c}",
                             name=f"sT{c}") for c in range(KT)]
              s_tiles = []
              for it in range(ITILES):
                  s_f32 = stage.tile([128, D], F32, tag="ld", name=f"sld{it}")
                  nc.gpsimd.dma_start(out=s_f32,
                                      in_=src[it * 128:(it + 1) * 128, :])
                  s_tiles.append(s_f32)
              for it in range(ITILES):
                  s_f32 = s_tiles[it]
                  sq = sqsc.tile([128, D], F32, tag="sq")
                  snorm = smalls.tile([128, 1], F32, tag="snorm")
                  nc.scalar.activation(out=sq, in_=s_f32, func=AF.Square,
                                       accum_out=snorm)
                  pre_t = pre.tile([128, D], BF16, tag="pre")
                  nc.scalar.activation(out=pre_t, in_=s_f32, func=AF.Copy,
                                       scale=2.0)
                  for c in range(KT):
                      pstt = pstr.tile([128, 128], BF16, tag="pst",
                                       name=f"spt{it}_{c}")
                      nc.tensor.transpose(pstt, pre_t[:, c * 128:(c + 1) * 128],
                                          ident)
                      nc.vector.tensor_copy(sT[c][:, it * 128:(it + 1) * 128],
                                            pstt)
                  _c = (it // 4) * 128 + 32 * (it % 4)
                  nc.vector.tensor_scalar_mul(scmb[:, _c:_c + 1], snorm, -1.0)

              for b in range(2):
                  pst_s = pstr.tile([128, 128], BF16, tag="pst",
                                    name=f"pst_s{b}")
                  nc.tensor.transpose(pst_s, scmb[:, b * 128:(b + 1) * 128],
                                      ident)
                  for q in range(4):
                      it = b * 4 + q
                      nc.scalar.copy(out=saugT[:, it * 128:(it + 1) * 128],
                                     in_=pst_s[32 * q:32 * q + 2, :])

              saugT
              # ---------- T side: software-pipelined groups ----------
              NMERGE = sum(((njb + 1) // 2 if PSUM_PAIRS else njb)
                           for njb in GROUP_JBS)
              mb = [mbp.tile([128, NMERGE * 8], F32, tag=f"mb{it}",
                             name=f"mb{it}") for it in range(ITILES)]

              def issue_loads(g, njb, jb0):
                  gjt = njb * 4
                  tiles = []
                  for jl in range(gjt):
                      jt = jb0 * 4 + jl
                      t_f32 = stage.tile([128, D], F32, tag="ld",
                                         name=f"tld{g}_{jl}")
                      nc.gpsimd.dma_start(out=t_f32,
                                          in_=tgt[jt * 128:(jt + 1) * 128, :])
                      tiles.append(t_f32)
                  return tiles

              def process_group(g, njb, jb0, tiles, mcol):
                  grows = njb * 512
                  gjt = grows // 128
                  jt0 = jb0 * 4
                  tbf = dram.tile([grows, D], BF16, tag=f"tbf{g}",
                                  name=f"tbf{g}")
                  tcmb = tTp.tile([128, 32 * gjt], BF16, tag="tcmb",
                                  name=f"tcmb{g}")
                  tnorm_g = tTp.tile([128, gjt], F32, tag="tnorm_g",
                                     name=f"tnorm_g{g}")
                  pe_path = g < PE_T_GROUPS
                  if pe_path:
                      tT = [tTp.tile([128, grows], BF16, tag=f"tT{c}",
                                     name=f"tT{c}_{g}") for c in range(KT)]
                  for jl in range(gjt):
                      jt = jt0 + jl
                      t_f32 = tiles[jl]
                      tq = sqsc.tile([128, D], F32, tag="sq")
                      nc.scalar.activation(out=tq, in_=t_f32, func=AF.Square,
                                           accum_out=tnorm_g[:, jl:jl + 1])
                      pre_t = pre.tile([128, D], BF16, tag="pre")
                      nc.scalar.activation(out=pre_t, in_=t_f32, func=AF.Copy,
                                           scale=bb_all[:, jt:jt + 1])
                      if pe_path:
                          for c in range(KT):
                              pstt = pstr.tile([128, 128], BF16, tag="pst",
                                               name=f"tpt{g}_{jl}_{c}")
                              nc.tensor.transpose(
                                  pstt, pre_t[:, c * 128:(c + 1) * 128], ident)
                              nc.vector.tensor_copy(
                                  tT[c][:, jl * 128:(jl + 1) * 128], pstt)
                      else:
                          nc.scalar.dma_start(
                              out=tbf[jl * 128:(jl + 1) * 128, :], in_=pre_t)

                  bn_g = tTp.tile([128, gjt], F32, tag="bn_g", name=f"bn_g{g}")
                  nc.vector.tensor_mul(bn_g, bb_all[:, jt0:jt0 + gjt], tnorm_g)
                  nbatch = (gjt + 3) // 4
                  nc.vector.memset(tcmb, 0.0)
                  tcmb4 = tcmb.rearrange("p (b q o) -> p b q o", q=4, o=32)
                  nc.vector.tensor_copy(
                      tcmb4[:, :, :, 0:1],
                      bb_all[:, jt0:jt0 + gjt].rearrange(
                          "p (b q one) -> p b q one", q=4, one=1))
                  nc.vector.tensor_copy(
                      tcmb4[:, :, :, 1:2],
                      bn_g.rearrange("p (b q one) -> p b q one", q=4, one=1))

                  taug = tTp.tile([2, grows], BF16, tag="taug", name=f"taug{g}")
                  for b in range(nbatch):
                      pst = pstr.tile([128, 128], BF16, tag="pst",
                                      name=f"pst{g}_{b}")
                      nc.tensor.transpose(pst, tcmb[:, b * 128:(b + 1) * 128],
                                          ident)
                      for q in range(4):
                          jl = b * 4 + q
                          nc.vector.tensor_copy(taug[:, jl * 128:(jl + 1) * 128],
                                                pst[32 * q:32 * q + 2, :])

                  if not pe_path:
                      tT = [tTp.tile([128, grows], BF16, tag=f"tT{c}",
                                     name=f"tT{c}_{g}") for c in range(KT)]
                      for c in range(KT):
                          nc.sync.dma_start(out=tT[c],
                                            in_=tbf[:, c * 128:(c + 1) * 128],
                                            transpose=True)

                  for it in range(ITILES):
                      pcol = mcol
                      pos = 0
                      npair = (njb + 1) // 2 if PSUM_PAIRS else njb
                      for pi in range(npair):
                          nhalf = (2 if pos + 1 < njb else 1) if PSUM_PAIRS else 1
                          ps2 = psum.tile([128, 512 * nhalf], F32, tag="ps")
                          for half in range(nhalf):
                              jbl = pos + half
                              pslice = ps2[:, half * 512:(half + 1) * 512]
                              for c in range(KT):
                                  nc.tensor.matmul(
                                      pslice,
                                      lhsT=sT[c][:, it * 128:(it + 1) * 128],
                                      rhs=tT[c][:, jbl * 512:(jbl + 1) * 512],
                                      start=(c == 0), stop=False)
                              nc.tensor.matmul(
                                  pslice,
                                  lhsT=saugT[:, it * 128:(it + 1) * 128],
                                  rhs=taug[:, jbl * 512:(jbl + 1) * 512],
                                  start=False, stop=True)
                          nc.vector.max(out=mb[it][:, pcol:pcol + 8], in_=ps2)
                          pcol += 8
                          pos += nhalf

              # run the pipeline with a 1-group skew
              jb0s = [0]
              mcols = [0]
              for njb in GROUP_JBS:
                  jb0s.append(jb0s[-1] + njb)
                  mcols.append(mcols[-1]
                               + ((njb + 1) // 2 if PSUM_PAIRS else njb) * 8)
              tiles_prev = None
              for g in range(len(GROUP_JBS) + 1):
                  if g < len(GROUP_JBS):
                      tiles = issue_loads(g, GROUP_JBS[g], jb0s[g])
                  if tiles_prev is not None:
                      pg = g - 1
                      process_group(pg, GROUP_JBS[pg], jb0s[pg], tiles_prev,
                                    mcols[pg])
                  if g < len(GROUP_JBS):
                      tiles_prev = tiles

              # ---------- deferred normalization factors (finalize only) ----------
              def bcast_norm_factor(full_sb, count, tag):
                  """r = count / (sum(full_sb) + EPS), broadcast to [128,1]."""
                  s1 = smalls.tile([128, 1], F32, tag=f"{tag}_s1")
                  nc.vector.tensor_reduce(out=s1, in_=full_sb, axis=AX.X,
                                          op=mybir.AluOpType.add)
                  ps1 = psum.tile([1, 1], F32, tag="ps")
                  nc.tensor.matmul(ps1, lhsT=s1, rhs=ones_col, start=True,
                                   stop=True)
                  sc = smalls.tile([1, 1], F32, tag=f"{tag}_sc")
                  nc.scalar.copy(out=sc, in_=ps1)
                  psb = psum.tile([128, 1], F32, tag="ps")
                  nc.tensor.matmul(psb, lhsT=ones_row, rhs=sc, start=True,
                                   stop=True)
                  r = const.tile([128, 1], F32, tag=f"{tag}_r")
                  nc.scalar.copy(out=r, in_=psb)
                  nc.vector.tensor_scalar_add(r, r, EPS)
                  nc.vector.reciprocal(r, r)
                  nc.vector.tensor_scalar_mul(r, r, float(count))
                  return r

              swsb = load_colmajor(sw, ITILES, "swsb")
              swfsb = load_colmajor(swf, N // 128, "swfsb")
              rs = bcast_norm_factor(swfsb, N, "rs")
              rt = bcast_norm_factor(twsb, M, "rt")
              rq = smalls.tile([128, 1], F32, tag="rq")
              nc.vector.tensor_mul(rq, rs, rt)
              naa_all = const.tile([128, ITILES], F32, tag="naa_all")
              nc.vector.tensor_scalar_mul(naa_all, swsb, rq[:, 0:1])
              nc.vector.tensor_mul(naa_all, naa_all, naa_all)
              nc.vector.tensor_scalar_mul(naa_all, naa_all, -1.0)

              # ---------- finalize: merge, sqrt(a * wd2'), hinge, row sums ----------
              for it in range(ITILES):
                  top8 = fin.tile([128, 8], F32, tag="top8")
                  nc.vector.max(out=top8, in_=mb[it])
                  nc.vector.tensor_scalar_min(top8[:, 0:TOPK], top8[:, 0:TOPK],
                                              0.0)
                  vals = fin.tile([128, TOPK], F32, tag="vals")
                  nc.scalar.activation(out=vals, in_=top8[:, 0:TOPK],
                                       func=AF.Sqrt, scale=naa_all[:, it:it + 1])
                  hout = fin.tile([128, TOPK], F32, tag="hout")
                  hsum = fin.tile([128, 1], F32, tag="hsum")
                  nc.scalar.activation(out=hout, in_=vals, func=AF.Relu,
                                       bias=hbias[:, 0:1], accum_out=hsum)
                  nc.sync.dma_start(
                      out=out[it].rearrange("(p one) -> p one", one=1), in_=hsum)

    nc.compile()
    return nc


def _get_nc():
    if "nc" not in _CACHE:
        _CACHE["nc"] = _build()
    return _CACHE["nc"]


def kernel(source, target, source_weights, target_weights, top_k):
    from concourse.bass_utils import run_bass_kernel_spmd

    assert int(top_k) == TOPK
    source = np.ascontiguousarray(np.asarray(source, dtype=np.float32))
    target = np.ascontiguousarray(np.asarray(target, dtype=np.float32))
    sw = np.ascontiguousarray(np.asarray(source_weights, dtype=np.float32))
    tw = np.ascontiguousarray(np.asarray(target_weights, dtype=np.float32))

    nc = _get_nc()
    in_maps = []
    for c in range(NCORES):
        in_maps.append({
            "src": np.ascontiguousarray(source[c * NSH:(c + 1) * NSH]),
            "sw": np.ascontiguousarray(sw[c * NSH:(c + 1) * NSH]),
            "swf": sw,
            "tgt": target,
            "tw": tw,
        })
    res = run_bass_kernel_spmd(nc, in_maps, list(range(NCORES)))
    total = 0.0
    for c in range(NCORES):
        total += float(np.sum(res.results[c]["partial"], dtype=np.float64))
    return np.float32(total / (N * TOPK))

